# revision 1
# baseline (speedup 1.0000x reference)
"""Trainium2 Bass kernel for nn_Attention_16612933501287.

Cross-attention block: c:(B=8,N=8,C=512,H=32,W=32), RMSNorm over C, fused
KV projection (512->1024), one query per (batch, head) attending over the
N=8 token axis at each spatial position, then output projection (512->512).

Sharding: data-parallel over B — one batch element per NeuronCore (8 cores).

Per-core dataflow (feature-major: channels on partitions, the 1024 spatial
positions on the free dim):
  host prep : fold g into Wkv; qv = emb[q]@Wq+bq; fold qv and the 1/sqrt(64)
              logit scale into a per-batch matrix Wd (512x8) so attention
              logits come straight out of a matmul; k is never materialized.
  n loop    : DMA c[n]; square (DVE/ACT/GPSIMD); ssq and logits accumulate
              across n into persistent PSUM tiles via one-hot-padded
              stationary weights; vraw = Wv.T@cp -> fp16 in SBUF.
  epilogue  : batched softmax (one Sqrt + one Exp -> only 2 ACT table
              loads); softmax denominator via an exact-fp32 selection
              matmul; w~ = e*r/sums in fp16; per-head replication via
              broadcast DMAs from a DRAM bounce (all issued upfront);
              vw = vraw*w~ (DVE fp16); sum over n via identity-matmul
              PSUM accumulation; output projection + bias; DMA out in
              (C,H,W) layout.
Big matmuls run as float32r (fp32 data, 1 PE cycle/row).
"""

import numpy as np

import concourse.bass as bass
import concourse.bacc as bacc
import concourse.mybir as mybir
import concourse.tile as tile
from concourse.bass_utils import run_bass_kernel_spmd

F32 = mybir.dt.float32
F16 = mybir.dt.float16
F32R = mybir.dt.float32r
AF = mybir.ActivationFunctionType

B, N, C, H, W = 8, 8, 512, 32, 32
NH, HS = 8, 64
P = H * W           # 1024 spatial positions per core
NCC = C // 128      # 4 contraction chunks
EPS = 1e-6


def r32(ap):
    return ap if ap.dtype == F32R else ap.bitcast(F32R)


def build_program():
    nc = bacc.Bacc()

    c_d = nc.declare_dram_parameter("c", [N, C, H, W], F32R, isOutput=False)
    wv_d = nc.declare_dram_parameter("wv", [128, NCC, 512], F32R, isOutput=False)
    # zero-padded logit weights: [k, cc, n, n*8+i] nonzero only at column n*8+i
    wdz_d = nc.declare_dram_parameter("wdz", [128, NCC, N, N * NH], F32R,
                                      isOutput=False)
    oh_d = nc.declare_dram_parameter("onehot", [128, N, N], F32R, isOutput=False)
    sel_d = nc.declare_dram_parameter("sel", [N * NH, NH], F32, isOutput=False)
    r8_d = nc.declare_dram_parameter("r8sel", [NH, 2, NH * NH], F32, isOutput=False)
    s64_d = nc.declare_dram_parameter("sel64", [N * NH, N, NCC, 128], F16,
                                      isOutput=False)
    wo_d = nc.declare_dram_parameter("wout", [128, NCC, 512], F16, isOutput=False)
    id_d = nc.declare_dram_parameter("ident", [128, 128], F16, isOutput=False)
    bo_d = nc.declare_dram_parameter("bout", [128, NCC], F32, isOutput=False)
    out_d = nc.declare_dram_parameter("out", [C, H, W], F32, isOutput=True)

    with tile.TileContext(nc) as tc:
        with (
            tc.tile_pool(name="consts", bufs=1) as consts,
            tc.tile_pool(name="store", bufs=1) as store,
            tc.tile_pool(name="smalls", bufs=1) as smalls,
            tc.tile_pool(name="osb_pool", bufs=2) as osb_pool,
            tc.tile_pool(name="ps_stat", bufs=1, space="PSUM") as ps_stat,
            tc.tile_pool(name="ps_big", bufs=2, space="PSUM") as ps_big,
        ):
            # loop-critical consts first (tiny oh so PE can start early);
            # wv/wdz loads are emitted inside n=0 after the first cp chunks,
            # epilogue-only weights after the loop.
            wdz_sb = consts.tile([128, NCC, N, N * NH], F32R)
            nc.sync.dma_start(out=wdz_sb[:, 0], in_=wdz_d[:, 0])
            wv_sb = consts.tile([128, NCC, 512], F32R)
            nc.sync.dma_start(out=wv_sb[:, 0], in_=wv_d[:, 0])
            oh_sb = consts.tile([128, N, N], F32R)
            nc.sync.dma_start(out=oh_sb, in_=oh_d[:])
            sel_sb = consts.tile([N * NH, NH], F32)
            r8_sb = consts.tile([NH, 2, NH * NH], F32)
            s64_sb = consts.tile([N * NH, N, NCC, 128], F16)
            wo_sb = consts.tile([128, NCC, 512], F16)
            id_sb = consts.tile([128, 128], F16)
            bo_sb = consts.tile([128, NCC], F32)

            # persistent accumulators / stores
            vraw_all = store.tile([128, N, NCC, P], F16)   # 8 MiB
            o_sb = store.tile([128, NCC, P], F16)
            ssq_ps = ps_stat.tile([N, P], F32)             # 2 banks, whole loop
            draw_ps = ps_stat.tile([N * NH, P], F32)       # 2 banks, whole loop

            # ================= main loop over token index n =================
            cp_ctx = tc.tile_pool(name="cp_pool", bufs=3)
            cp_pool = cp_ctx.__enter__()
            sq_ctx = tc.tile_pool(name="sq_pool", bufs=1)
            sq_pool = sq_ctx.__enter__()
            for n in range(N):
                cp = cp_pool.tile([128, NCC, P], F32R)
                if n == 0:
                    # per-cc loads interleaved with the weights they unblock
                    for cc in range(NCC):
                        nc.sync.dma_start(
                            out=cp[:, cc, :],
                            in_=c_d[:].rearrange(
                                "n (cc k) h w -> n cc k (h w)", k=128)[n, cc],
                        )
                        if cc < NCC - 1:
                            nc.sync.dma_start(out=wdz_sb[:, cc + 1],
                                              in_=wdz_d[:, cc + 1])
                            nc.sync.dma_start(out=wv_sb[:, cc + 1],
                                              in_=wv_d[:, cc + 1])
                else:
                    nc.sync.dma_start(
                        out=cp,
                        in_=c_d[:].rearrange(
                            "n (cc k) h w -> n k cc (h w)", k=128)[n],
                    )

                def emit_draw(n=n, cp=cp):
                    for cc in range(NCC):
                        for h in range(2):
                            nc.tensor.matmul(
                                draw_ps[:, h * 512:(h + 1) * 512],
                                r32(wdz_sb[:, cc, n, :]),
                                r32(cp[:, cc, h * 512:(h + 1) * 512]),
                                start=(n == 0 and cc == 0),
                                stop=(n == N - 1 and cc == NCC - 1),
                            )

                def emit_vraw(n=n, cp=cp):
                    # cc-outer / h-inner: one weight load serves both halves
                    for ck in range(NCC):
                        v_ps = ps_big.tile([128, P], F32, tag="pair",
                                           name="v_ps")
                        for cc in range(NCC):
                            for h in range(2):
                                nc.tensor.matmul(
                                    v_ps[:, h * 512:(h + 1) * 512],
                                    r32(wv_sb[:, cc, ck * 128:(ck + 1) * 128]),
                                    r32(cp[:, cc, h * 512:(h + 1) * 512]),
                                    start=(cc == 0),
                                    stop=(cc == NCC - 1),
                                )
                        nc.scalar.copy(out=vraw_all[:, n, ck, :], in_=v_ps)

                def emit_ssq(n=n, cp=cp):
                    sq = sq_pool.tile([128, NCC, P], F32R, name="sq")
                    nc.vector.tensor_mul(out=sq[:, 0, :], in0=cp[:, 0, :], in1=cp[:, 0, :])
                    nc.gpsimd.tensor_mul(out=sq[:, 1, :], in0=cp[:, 1, :], in1=cp[:, 1, :])
                    nc.gpsimd.tensor_mul(out=sq[:, 2, :], in0=cp[:, 2, :], in1=cp[:, 2, :])
                    nc.gpsimd.tensor_mul(out=sq[:, 3, :], in0=cp[:, 3, :], in1=cp[:, 3, :])
                    # pre-sum the 4 chunks so ssq needs 2 matmuls/n, not 8
                    sqs = sq_pool.tile([128, P], F32R, name="sqs")
                    nc.vector.tensor_add(out=sqs, in0=sq[:, 0, :], in1=sq[:, 1, :])
                    nc.gpsimd.tensor_add(out=sq[:, 2, :], in0=sq[:, 2, :], in1=sq[:, 3, :])
                    nc.vector.tensor_add(out=sqs, in0=sqs, in1=sq[:, 2, :])
                    for h in range(2):
                        nc.tensor.matmul(
                            ssq_ps[:, h * 512:(h + 1) * 512],
                            r32(oh_sb[:, n, :]),
                            r32(sqs[:, h * 512:(h + 1) * 512]),
                            start=(n == 0),
                            stop=(n == N - 1),
                        )

                if n < N - 2:
                    # stats are epilogue-only: emit them last
                    emit_draw(); emit_vraw(); emit_ssq()
                elif n == N - 2:
                    # defer this vraw until after n=7's stats (loop tail)
                    emit_ssq(); emit_draw()
                    deferred_vraw = emit_vraw
                else:
                    # n=7: stats first, then both deferred vraws — the
                    # softmax chain hides under ~14us of vraw matmuls
                    emit_ssq(); emit_draw()
                    deferred_vraw(); emit_vraw()
            sq_ctx.__exit__(None, None, None)
            cp_ctx.__exit__(None, None, None)

            # ======================== epilogue ========================
            # epilogue-only weights (land during the loop's DMA slack)
            nc.sync.dma_start(out=sel_sb, in_=sel_d[:])
            nc.sync.dma_start(out=r8_sb, in_=r8_d[:])
            nc.sync.dma_start(out=s64_sb, in_=s64_d[:])
            nc.sync.dma_start(out=wo_sb, in_=wo_d[:])
            nc.sync.dma_start(out=id_sb, in_=id_d[:])
            nc.sync.dma_start(out=bo_sb, in_=bo_d[:])

            # softmax chain, split into independent h-halves so the two
            # halves pipeline through ACT/DVE/PE (halves the serial latency)
            eps_sb = smalls.tile([N, 1], F32)
            nc.vector.memset(eps_sb, EPS)
            rt = smalls.tile([N, P], F32)
            r_all = rt
            rrep = smalls.tile([N * NH, P], F32)
            e_all = smalls.tile([N * NH, P], F32)
            rsum = smalls.tile([NH, P], F32)
            srep = smalls.tile([N * NH, P], F32)
            wt = smalls.tile([N * NH, P], F16)
            for h in range(2):
                hs_ = slice(h * 512, (h + 1) * 512)
                # r = 1/sqrt(ssq/C + eps)
                nc.scalar.activation(out=rt[:, hs_], in_=ssq_ps[:, hs_],
                                     func=AF.Sqrt, scale=1.0 / C, bias=eps_sb)
                nc.vector.reciprocal_approx_fast(out=r_all[:, hs_], in_=rt[:, hs_])
                # rrep[n*8+i] = r_all[n] via selection matmul (exact fp32)
                rr_ps = ps_big.tile([N * NH, 512], F32, tag="pair", name="rr_ps")
                nc.tensor.matmul(rr_ps, r8_sb[:, 0, :], r_all[:, hs_],
                                 start=True, stop=True)
                nc.scalar.copy(out=rrep[:, hs_], in_=rr_ps)
                # dots = draw * r ; e = exp(dots)
                nc.vector.tensor_mul(out=e_all[:, hs_], in0=draw_ps[:, hs_],
                                     in1=rrep[:, hs_])
                nc.scalar.activation(out=e_all[:, hs_], in_=e_all[:, hs_],
                                     func=AF.Exp)
                # softmax denominator (exact-fp32 matmul), reciprocal
                s_ps = ps_big.tile([NH, 512], F32, tag="pair", name="s_ps")
                nc.tensor.matmul(s_ps, sel_sb, e_all[:, hs_],
                                 start=True, stop=True)
                nc.vector.reciprocal_approx_fast(out=rsum[:, hs_], in_=s_ps)
                sr_ps = ps_big.tile([N * NH, 512], F32, tag="pair", name="sr_ps")
                nc.tensor.matmul(sr_ps, r8_sb[:, 1, :], rsum[:, hs_],
                                 start=True, stop=True)
                nc.scalar.copy(out=srep[:, hs_], in_=sr_ps)
                # w~ = e * r / sums  -> fp16
                nc.vector.tensor_mul(out=e_all[:, hs_], in0=e_all[:, hs_],
                                     in1=rrep[:, hs_])
                nc.vector.tensor_mul(out=wt[:, hs_], in0=e_all[:, hs_],
                                     in1=srep[:, hs_])

            with (
                tc.tile_pool(name="wrep_pool", bufs=4) as wrep_pool,
                tc.tile_pool(name="vw_pool", bufs=2) as vw_pool,
            ):
                # o = sum_n vraw * w~rep via identity-matmul PSUM accumulation;
                # per-head replication via selection matmuls from wt (on-chip)
                for ck in range(NCC):
                    # o-accumulator reuses the (now idle) stats PSUM banks so
                    # ps_big's 4 slots stay free for the wrep pipeline
                    on_ps = ps_stat.tile(
                        [128, P], F32, name=f"on_ps_{ck}",
                        tag=("ssq_ps" if ck % 2 == 0 else "draw_ps"))
                    for n in range(N):
                        vw = vw_pool.tile([128, P], F16)
                        wr_ps = ps_big.tile([128, P], F32, tag="pair")
                        for h in range(2):
                            nc.tensor.matmul(
                                wr_ps[:, h * 512:(h + 1) * 512],
                                s64_sb[:, n, ck, :],
                                wt[:, h * 512:(h + 1) * 512],
                                start=True, stop=True)
                        nc.vector.tensor_mul(
                            out=vw, in0=vraw_all[:, n, ck, :], in1=wr_ps)
                        for h in range(2):
                            nc.tensor.matmul(
                                on_ps[:, h * 512:(h + 1) * 512],
                                id_sb,
                                vw[:, h * 512:(h + 1) * 512],
                                start=(n == 0),
                                stop=(n == N - 1),
                            )
                    for h in range(2):
                        nc.scalar.copy(
                            out=o_sb[:, ck, h * 512:(h + 1) * 512],
                            in_=on_ps[:, h * 512:(h + 1) * 512]
                        )

                # out = Wout.T @ o + bout
                for do in range(NCC):
                    ot_sb = osb_pool.tile([128, P], F32)
                    ot_ps = ps_big.tile([128, P], F32, tag="pair")
                    for h in range(2):
                        for di in range(NCC):
                            nc.tensor.matmul(
                                ot_ps[:, h * 512:(h + 1) * 512],
                                wo_sb[:, di, do * 128:(do + 1) * 128],
                                o_sb[:, di, h * 512:(h + 1) * 512],
                                start=(di == 0),
                                stop=(di == NCC - 1),
                            )
                    nc.scalar.activation(
                        out=ot_sb, in_=ot_ps,
                        func=AF.Identity, bias=bo_sb[:, do:do + 1],
                    )
                    nc.sync.dma_start(
                        out=out_d[:].rearrange(
                            "(do k) h w -> do k (h w)", k=128)[do],
                        in_=ot_sb,
                    )

    nc.finalize()
    return nc


_CACHE = {}


def _get_nc():
    if "nc" not in _CACHE:
        _CACHE["nc"] = build_program()
    return _CACHE["nc"]


def _prep_inputs(q, c, emb, Wq, bq, Wkv, Wout, bout, g):
    q = np.asarray(q)
    c = np.asarray(c, dtype=np.float32)
    emb = np.asarray(emb, dtype=np.float32)
    Wq = np.asarray(Wq, dtype=np.float32)
    bq = np.asarray(bq, dtype=np.float32)
    Wkv = np.asarray(Wkv, dtype=np.float32)
    Wout = np.asarray(Wout, dtype=np.float32)
    bout = np.asarray(bout, dtype=np.float32)
    g = np.asarray(g, dtype=np.float32)

    qv = emb[q] @ Wq + bq                                   # (B, 512)
    qvs = qv.reshape(B, NH, HS).astype(np.float32) * np.float32(HS ** -0.5)
    Wkv_g = (g[:, None] * Wkv).astype(np.float32)
    Wk3 = Wkv_g[:, :C].reshape(C, NH, HS)
    Wv = np.ascontiguousarray(Wkv_g[:, C:])                 # (512, 512)
    Wd = np.einsum('chs,bhs->bch', Wk3, qvs).astype(np.float32)  # (B, 512, 8)

    wv_host = np.ascontiguousarray(
        Wv.reshape(NCC, 128, 512).transpose(1, 0, 2))       # [k, cc, dv]
    # zero-padded draw weights: [b, k, cc, n, m] = Wd at m = n*8+i
    wdz = np.zeros((B, 128, NCC, N, N * NH), np.float32)
    wd4 = Wd.reshape(B, NCC, 128, NH).transpose(0, 2, 1, 3)  # [b, k, cc, i]
    for n in range(N):
        wdz[:, :, :, n, n * NH:(n + 1) * NH] = wd4
    wout_host = np.ascontiguousarray(
        Wout.reshape(NCC, 128, 512).transpose(1, 0, 2)).astype(np.float16)
    onehot = np.zeros((128, N, N), np.float32)
    for n in range(N):
        onehot[:, n, n] = 1.0
    sel = np.zeros((N * NH, NH), np.float32)
    for n in range(N):
        for i in range(NH):
            sel[n * NH + i, i] = 1.0
    # r8sel[:, 0]: rrep (out row n*8+i <- r row n); r8sel[:, 1]: srep (<- rsum row i)
    r8sel = np.zeros((NH, 2, NH * NH), np.float32)
    for n in range(N):
        for i in range(NH):
            r8sel[n, 0, n * NH + i] = 1.0
            r8sel[i, 1, n * NH + i] = 1.0
    # sel64[kk, n, ck, m] = 1 iff kk == n*8 + 2*ck + m//64
    sel64 = np.zeros((N * NH, N, NCC, 128), np.float16)
    for n in range(N):
        for ck in range(NCC):
            for j in range(2):
                sel64[n * NH + 2 * ck + j, n, ck, j * 64:(j + 1) * 64] = 1.0
    ident = np.eye(128, dtype=np.float16)
    bout_host = np.ascontiguousarray(bout.reshape(NCC, 128).T)  # [k, do]

    in_maps = []
    for b in range(B):
        in_maps.append({
            "c": np.ascontiguousarray(c[b]),
            "wv": wv_host,
            "wdz": np.ascontiguousarray(wdz[b]),
            "onehot": onehot,
            "sel": sel,
            "r8sel": r8sel,
            "sel64": sel64,
            "wout": wout_host,
            "ident": ident,
            "bout": bout_host,
        })
    return in_maps


def kernel(**inputs) -> np.ndarray:
    nc = _get_nc()
    in_maps = _prep_inputs(**inputs)
    res = run_bass_kernel_spmd(nc, in_maps, list(range(B)))
    return np.stack([res.results[b]["out"] for b in range(B)], axis=0)


if __name__ == "__main__":
    nc = build_program()
    print("program built ok")



# revision 48
# speedup vs baseline: 1.0636x; 1.0636x over previous
"""Trainium2 Bass kernel for nn_Attention_16612933501287.

Cross-attention block: c:(B=8,N=8,C=512,H=32,W=32), RMSNorm over C, fused
KV projection (512->1024), one query per (batch, head) attending over the
N=8 token axis at each spatial position, then output projection (512->512).

Sharding: data-parallel over B -- one batch element per NeuronCore (8 cores).

Fully fused single-pass loop over the token axis n (feature-major layout:
channels on partitions, the 1024 spatial positions on the free dim):

  host prep : fold g into Wkv; qv = emb[q]@Wq+bq; fold qv and the 1/sqrt(64)
              logit scale into per-batch logit weights Wd (512x8). The v
              output dim is permuted head-interleaved (dv = H*64 + j*4 + ck)
              so every 128-row vraw chunk has the same partition->head map
              (m//16), letting ONE selection matmul replicate softmax weights
              for all 4 chunks. Wout rows are permuted to match.
  per token : draw_{n+1} = Wd.T@cp runs at the top of iteration n and
              ssq_{n+2} (channel sum-sq via an all-ones [128,8] stationary,
              replicated onto 8 rows) at its tail, so the softmax-partial
              chain (ln -> r=exp(-.5x) -> dots -> e=exp -> wt=e*r, all from
              ONE manually pre-loaded ACT table: no table reloads) finishes
              a full iteration before its token's vraw. vraw streams as 8
              [128,512] chunks through a 3x1-bank PSUM pipe; each chunk is
              weighted on DVE (vw = v_ps * wrep16) and accumulated into
              SBUF fp16 by Pool. wrep comes from one fp16 selection matmul
              per half, bounced PSUM->SBUF fp16 by ACT. Chain DVE ops are
              emitted in 512-halves spread across the vraw groups so the
              greedy per-engine scheduler never blocks the PSUM drain.
  final iter: sums is complete one iteration early, so 1/sums, the
              pre-normalized weights wt7*rsum, and srep (row-replicated
              1/sums) are built in iteration 6's slack; iteration 7 lands
              fully-normalized chunks directly on onorm (o_acc*srep is
              folded in per chunk), letting the output projection, bias
              (split ACT/DVE halves) and out-DMA halves start immediately.
Big matmuls run as float32r (fp32 data, 1 PE cycle/row at >=256 cols).
"""

import numpy as np

import concourse.bass as bass
import concourse.bacc as bacc
import concourse.mybir as mybir
import concourse.tile as tile
from concourse.bass_utils import run_bass_kernel_spmd

F32 = mybir.dt.float32
F16 = mybir.dt.float16
F32R = mybir.dt.float32r
AF = mybir.ActivationFunctionType

B, N, C, H, W = 8, 8, 512, 32, 32
NH, HS = 8, 64
P = H * W           # 1024 spatial positions per core
NCC = C // 128      # 4 contraction chunks
EPS = 1e-6


def r32(ap):
    return ap if ap.dtype == F32R else ap.bitcast(F32R)


def build_program():
    nc = bacc.Bacc()

    c_d = nc.declare_dram_parameter("c", [N, C, H, W], F32R, isOutput=False)
    wd_d = nc.declare_dram_parameter("wd", [128, NCC, NH], F32R, isOutput=False)
    wv_d = nc.declare_dram_parameter("wv", [128, NCC, 512], F32R, isOutput=False)
    sel8_d = nc.declare_dram_parameter("sel8", [NH, 128], F16, isOutput=False)
    ones8_d = nc.declare_dram_parameter("ones8", [128, NH], F32R, isOutput=False)
    wo_d = nc.declare_dram_parameter("wout", [128, NCC, 512], F16, isOutput=False)
    bo_d = nc.declare_dram_parameter("bout", [128, NCC], F32, isOutput=False)
    out_d = nc.declare_dram_parameter("out", [C, H, W], F16, isOutput=True)

    with tile.TileContext(nc) as tc:
        with (
            tc.tile_pool(name="consts", bufs=1) as consts,
            tc.tile_pool(name="store", bufs=1) as store,
            tc.tile_pool(name="cp_pool", bufs=4) as cp_pool,
            tc.tile_pool(name="sq_pool", bufs=2) as sq_pool,
            tc.tile_pool(name="chain", bufs=2) as chain,
            tc.tile_pool(name="vw_pool", bufs=6) as vw_pool,
            tc.tile_pool(name="osb_pool", bufs=4) as osb_pool,
            tc.tile_pool(name="ps_v", bufs=3, space="PSUM") as ps_v,
            tc.tile_pool(name="ps_draw", bufs=1, space="PSUM") as ps_draw,
            tc.tile_pool(name="ps_ssq", bufs=1, space="PSUM") as ps_ssq,
            tc.tile_pool(name="ps_wrep", bufs=1, space="PSUM") as ps_wrep,
        ):
            # REP_BODY_BEGIN
            # pre-load the one ACT table that serves every function used
            # here (ln/exp/square/copy/identity); the auto-inserter is
            # greedy-per-func and would otherwise ping-pong tables per token
            nc.scalar.add_instruction(mybir.InstLoadActFuncSet(
                name=nc.get_next_instruction_name(), ins=[], outs=[],
                act_func_set_id=6))
            cp = {}
            cp[0] = cp_pool.tile([128, NCC, P], F32R, name="cp")
            cp[1] = cp_pool.tile([128, NCC, P], F32R, name="cp")
            # token 0 streams in half-cc granules so squares/stats start on
            # the first arrival; weights for vraw_0 come before token 1
            c_r = c_d[:].rearrange("n (cc k) h w -> n cc k (h w)", k=128)
            nc.sync.dma_start(out=cp[0][:, 0, 0:512], in_=c_r[0, 0][:, 0:512])
            wd_sb = consts.tile([128, NCC, NH], F32R)
            nc.sync.dma_start(out=wd_sb, in_=wd_d[:])
            sel8_sb = consts.tile([NH, 128], F16)
            nc.sync.dma_start(out=sel8_sb, in_=sel8_d[:])
            ones8_sb = consts.tile([128, NH], F32R)
            nc.sync.dma_start(out=ones8_sb, in_=ones8_d[:])
            nc.sync.dma_start(out=cp[0][:, 0, 512:1024],
                              in_=c_r[0, 0][:, 512:1024])
            for cc in range(1, NCC):
                for h in range(2):
                    hs_ = slice(h * 512, (h + 1) * 512)
                    nc.sync.dma_start(out=cp[0][:, cc, hs_],
                                      in_=c_r[0, cc][:, hs_])
            wv_sb = consts.tile([128, NCC, 512], F32R)
            for ck in range(NCC):
                nc.sync.dma_start(out=wv_sb[:, :, ck * 128:(ck + 1) * 128],
                                  in_=wv_d[:, :, ck * 128:(ck + 1) * 128])
                nc.sync.dma_start(out=cp[1][:, ck, :], in_=c_r[1, ck])
            wo_sb = consts.tile([128, NCC, 512], F16)
            bo_sb = consts.tile([128, NCC], F32)

            eps_sb = consts.tile([NH, 1], F32)
            nc.vector.memset(eps_sb, EPS)

            o_acc = store.tile([128, NCC, P], F16)
            sums = store.tile([NH, P], F32)

            sq = {}
            draw_ps = {}
            ssq_ps = {}
            wt = {}

            def emit_squares(n, half=False):
                sq[n] = sq_pool.tile([128, NCC, P], F32R, name="sq")
                for cc in range(NCC):
                    if half:
                        for h in range(2):
                            hs_ = slice(h * 512, (h + 1) * 512)
                            nc.scalar.activation(
                                out=sq[n][:, cc, hs_],
                                in_=cp[n][:, cc, hs_].bitcast(F32),
                                func=AF.Square)
                    else:
                        nc.scalar.activation(
                            out=sq[n][:, cc, :],
                            in_=cp[n][:, cc, :].bitcast(F32),
                            func=AF.Square)

            def emit_draw(n):
                # logits draw_n = Wd.T @ cp_n
                draw_ps[n] = ps_draw.tile([NH, P], F32, name="draw_ps")
                for cc in range(NCC):
                    for h in range(2):
                        hs_ = slice(h * 512, (h + 1) * 512)
                        nc.tensor.matmul(
                            draw_ps[n][:, hs_],
                            wd_sb[:, cc, :],
                            cp[n][:, cc, hs_],
                            start=(cc == 0), stop=(cc == NCC - 1),
                        )

            def emit_ssq(n):
                # channel sum-of-squares, replicated onto 8 rows by the
                # all-ones stationary (no later row-broadcast needed)
                ssq_ps[n] = ps_ssq.tile([NH, P], F32, name="ssq_ps")
                for cc in range(NCC):
                    for h in range(2):
                        hs_ = slice(h * 512, (h + 1) * 512)
                        nc.tensor.matmul(
                            ssq_ps[n][:, hs_],
                            ones8_sb,
                            sq[n][:, cc, hs_],
                            start=(cc == 0), stop=(cc == NCC - 1),
                        )

            r8 = {}
            e8 = {}

            def chain_ab(n):
                # r = rsqrt(ssq/C + eps) via exp(-0.5 ln(.)) -- both funcs
                # live in ACT table 6, so no table reloads; dots = draw*r.
                # DVE ops go in 512-halves so they can slot between vw
                # drains without stalling the PSUM pipe.
                lnm = chain.tile([NH, P], F32, name="lnm")
                nc.scalar.activation(out=lnm, in_=ssq_ps[n], func=AF.Ln,
                                     scale=1.0 / C, bias=eps_sb)
                r8[n] = chain.tile([NH, P], F32, name="r8")
                nc.scalar.activation(out=r8[n], in_=lnm, func=AF.Exp,
                                     scale=-0.5)
                dots = chain.tile([NH, P], F32, name="dots")
                for h in range(2):
                    hs_ = slice(h * 512, (h + 1) * 512)
                    nc.vector.tensor_mul(out=dots[:, hs_],
                                         in0=draw_ps[n][:, hs_],
                                         in1=r8[n][:, hs_])
                e8[n] = chain.tile([NH, P], F32, name="e8")
                nc.scalar.activation(out=e8[n], in_=dots, func=AF.Exp)

            def chain_wt(n, h):
                # wt = e*r (fp16) for the replication matmul
                if h == 0:
                    wt[n] = chain.tile([NH, P], F16, name="wt")
                hs_ = slice(h * 512, (h + 1) * 512)
                nc.vector.tensor_mul(out=wt[n][:, hs_],
                                     in0=e8[n][:, hs_],
                                     in1=r8[n][:, hs_])

            def chain_sums(n, h, eng=None):
                hs_ = slice(h * 512, (h + 1) * 512)
                if n == 0:
                    nc.vector.tensor_copy(out=sums[:, hs_],
                                          in_=e8[n][:, hs_])
                else:
                    (eng or nc.vector).tensor_add(out=sums[:, hs_],
                                                  in0=sums[:, hs_],
                                                  in1=e8[n][:, hs_])

            def chain_d(n):
                for h in range(2):
                    chain_wt(n, h)
                for h in range(2):
                    chain_sums(n, h)

            def emit_vraw(n, wtn=None, post_wrep=None, per_group=None):
                # per-head replication of wt: one fp16 selection matmul per
                # half through a 1-bank PSUM slot, bounced to SBUF by ACT
                # (which has slack) so the vraw pipe gets 3 PSUM banks
                final = n == N - 1
                wrep_sb = store.tile([128, P], F16, name="wrep_sb", bufs=2)
                for h in range(2):
                    hs_ = slice(h * 512, (h + 1) * 512)
                    wrep_ps = ps_wrep.tile([128, 512], F32, name="wrep_ps")
                    nc.tensor.matmul(wrep_ps, sel8_sb,
                                     (wtn if wtn is not None else wt[n])[:, hs_],
                                     start=True, stop=True)
                    nc.scalar.copy(out=wrep_sb[:, hs_], in_=wrep_ps)
                if post_wrep is not None:
                    post_wrep()
                for ck in range(NCC):
                    for h in range(2):
                        hs_ = slice(h * 512, (h + 1) * 512)
                        v_ps = ps_v.tile([128, 512], F32, name="v_ps")
                        for cc in range(NCC):
                            nc.tensor.matmul(
                                v_ps,
                                wv_sb[:, cc, ck * 128:(ck + 1) * 128],
                                cp[n][:, cc, hs_],
                                start=(cc == 0), stop=(cc == NCC - 1),
                            )
                        if n == 0:
                            nc.vector.tensor_mul(
                                out=o_acc[:, ck, hs_], in0=v_ps,
                                in1=wrep_sb[:, hs_])
                        elif final or (n == N - 2):
                            # ACT-drained path: PSUM bounced to fp16 by ACT
                            # (idle at the tail), fp16 2x multiply on DVE.
                            # Used for the whole final token and for the
                            # last groups of iteration 6, whose DVE drains
                            # would otherwise stall the final iteration.
                            v7 = vw_pool.tile([128, 512], F16, name="vw")
                            nc.scalar.copy(out=v7, in_=v_ps)
                            vwp = vw_pool.tile([128, 512], F16, name="vw")
                            nc.vector.tensor_mul(
                                out=vwp, in0=v7, in1=wrep_sb[:, hs_])
                            if final:
                                (nc.gpsimd if ck % 2 == 0 else
                                 nc.vector).tensor_add(
                                    out=onorm[:, ck, hs_],
                                    in0=onorm[:, ck, hs_], in1=vwp)
                            else:
                                nc.gpsimd.tensor_add(
                                    out=o_acc[:, ck, hs_],
                                    in0=o_acc[:, ck, hs_], in1=vwp)
                        else:
                            vw = vw_pool.tile([128, 512], F16, name="vw")
                            nc.vector.tensor_mul(
                                out=vw, in0=v_ps, in1=wrep_sb[:, hs_])
                            nc.gpsimd.tensor_add(
                                out=o_acc[:, ck, hs_],
                                in0=o_acc[:, ck, hs_], in1=vw)
                        if per_group is not None:
                            per_group(ck * 2 + h)

            # ---------------- prologue: token 0 stats + chain -------------
            emit_squares(0, half=True)
            emit_draw(0)
            emit_ssq(0)
            chain_ab(0)
            chain_d(0)
            emit_squares(1)

            # ---------------- main loop over token index n ----------------
            # steady-state PE order per iteration: draw_{n+1}, wrep_n,
            # vraw_n chunks, ssq_{n+2}; the softmax chain for token n+1
            # spans the whole iteration with ~6us of slack.
            onorm = store.tile([128, NCC, P], F16)
            srep_sb = store.tile([128, P], F16)
            for n in range(N):
                if n + 2 < N:
                    # per-cc granules: squares/stats of token n+2 can start
                    # on first arrival instead of the full 2MB completion
                    cp[n + 2] = cp_pool.tile([128, NCC, P], F32R, name="cp")
                    for cc in range(NCC):
                        nc.sync.dma_start(out=cp[n + 2][:, cc, :],
                                          in_=c_r[n + 2, cc])
                if n + 1 < N:
                    emit_draw(n + 1)
                if n == 0:
                    emit_ssq(1)
                if n + 1 < N:
                    chain_ab(n + 1)
                if n + 1 < N and n != N - 2:
                    def mk_pg(nn):
                        def pg(g):
                            if g == 1:
                                chain_wt(nn, 0)
                            elif g == 2:
                                chain_wt(nn, 1)
                            elif g == 3:
                                chain_sums(nn, 0)
                            elif g == 4:
                                chain_sums(nn, 1)
                        return pg
                    per_group = mk_pg(n + 1)
                elif n == N - 2:
                    # spread the transition cluster (wt_7, sums_7 on Pool,
                    # rsum halves, wt7n = wt_7*rsum) across the iteration so
                    # the final iteration starts with a drained DVE
                    rsum = chain.tile([NH, P], F32, name="rsum")
                    rsum16 = chain.tile([NH, P], F16, name="rsum16")
                    wt7n = chain.tile([NH, P], F16, name="wt7n")

                    def per_group(g):
                        nn = N - 1
                        if g == 1:
                            chain_wt(nn, 0)
                        elif g == 2:
                            chain_wt(nn, 1)
                            chain_sums(nn, 0, eng=nc.gpsimd)
                        elif g == 3:
                            chain_sums(nn, 1, eng=nc.gpsimd)
                            nc.vector.reciprocal_approx_fast(
                                out=rsum[:, 0:512], in_=sums[:, 0:512])
                        elif g == 4:
                            nc.vector.reciprocal_approx_fast(
                                out=rsum[:, 512:1024], in_=sums[:, 512:1024])
                            nc.vector.tensor_copy(out=rsum16[:, 0:512],
                                                  in_=rsum[:, 0:512])
                        elif g == 5:
                            nc.vector.tensor_copy(out=rsum16[:, 512:1024],
                                                  in_=rsum[:, 512:1024])
                            nc.vector.tensor_mul(out=wt7n[:, 0:512],
                                                 in0=wt[nn][:, 0:512],
                                                 in1=rsum[:, 0:512])
                        elif g == 6:
                            nc.vector.tensor_mul(out=wt7n[:, 512:1024],
                                                 in0=wt[nn][:, 512:1024],
                                                 in1=rsum[:, 512:1024])
                if n == N - 1:
                    def post_wrep():
                        for h in range(2):
                            hs_ = slice(h * 512, (h + 1) * 512)
                            srep_ps = ps_wrep.tile([128, 512], F32,
                                                   name="srep",
                                                   tag="wrep_ps")
                            nc.tensor.matmul(srep_ps, sel8_sb,
                                             rsum16[:, hs_],
                                             start=True, stop=True)
                            nc.scalar.copy(out=srep_sb[:, hs_], in_=srep_ps)
                        for ck in range(NCC):
                            for h2 in range(2):
                                hs2 = slice(h2 * 512, (h2 + 1) * 512)
                                nc.vector.tensor_mul(
                                    out=onorm[:, ck, hs2],
                                    in0=o_acc[:, ck, hs2],
                                    in1=srep_sb[:, hs2])
                    emit_vraw(n, wtn=wt7n, post_wrep=post_wrep)
                else:
                    emit_vraw(n, per_group=per_group)
                if n + 2 < N:
                    emit_squares(n + 2)
                    emit_ssq(n + 2)
                if n == 2:
                    # epilogue-only weights: keep them off the congested
                    # early DMA queue
                    nc.sync.dma_start(out=wo_sb, in_=wo_d[:])
                    nc.sync.dma_start(out=bo_sb, in_=bo_d[:])

            # ============ epilogue: output projection + bias ============
            # half-tiles through the (now idle) 1-bank v_ps ring so each
            # half's accumulation group closes independently and its bias
            # (ACT/DVE alternating) + out-DMA fire immediately
            for do in range(NCC):
                ot_sb = osb_pool.tile([128, P], F16)
                for h in range(2):
                    hs_ = slice(h * 512, (h + 1) * 512)
                    ot_ps = ps_v.tile([128, 512], F32, name="v_ps")
                    for di in range(NCC):
                        nc.tensor.matmul(
                            ot_ps,
                            wo_sb[:, di, do * 128:(do + 1) * 128],
                            onorm[:, di, hs_],
                            start=(di == 0), stop=(di == NCC - 1),
                        )
                    if (do * 2 + h) % 2 == 0:
                        nc.scalar.activation(
                            out=ot_sb[:, hs_], in_=ot_ps,
                            func=AF.Identity, bias=bo_sb[:, do:do + 1],
                        )
                    else:
                        nc.vector.tensor_scalar_add(
                            out=ot_sb[:, hs_], in0=ot_ps,
                            scalar1=bo_sb[:, do:do + 1],
                        )
                    nc.sync.dma_start(
                        out=out_d[:].rearrange(
                            "(do k) h w -> do k (h w)", k=128)[do, :, hs_],
                        in_=ot_sb[:, hs_],
                    )
            # REP_BODY_END

    nc.finalize()
    return nc


_CACHE = {}


def _get_nc():
    if "nc" not in _CACHE:
        _CACHE["nc"] = build_program()
    return _CACHE["nc"]


def _perm():
    # dv index held by (chunk ck, partition m): head-uniform interleave
    m = np.arange(128)
    ck = np.arange(NCC)[:, None]
    return (m[None, :] // 16) * 64 + (m[None, :] % 16) * 4 + ck  # [ck, m]


def _prep_inputs(q, c, emb, Wq, bq, Wkv, Wout, bout, g):
    q = np.asarray(q)
    c = np.asarray(c, dtype=np.float32)
    emb = np.asarray(emb, dtype=np.float32)
    Wq = np.asarray(Wq, dtype=np.float32)
    bq = np.asarray(bq, dtype=np.float32)
    Wkv = np.asarray(Wkv, dtype=np.float32)
    Wout = np.asarray(Wout, dtype=np.float32)
    bout = np.asarray(bout, dtype=np.float32)
    g = np.asarray(g, dtype=np.float32)

    qv = emb[q] @ Wq + bq                                   # (B, 512)
    qvs = qv.reshape(B, NH, HS).astype(np.float32) * np.float32(HS ** -0.5)
    Wkv_g = (g[:, None] * Wkv).astype(np.float32)
    Wk3 = Wkv_g[:, :C].reshape(C, NH, HS)
    Wv = np.ascontiguousarray(Wkv_g[:, C:])                 # (512, 512)
    Wd = np.einsum('chs,bhs->bch', Wk3, qvs).astype(np.float32)  # (B, 512, 8)

    perm = _perm()                                          # [ck, m]
    # wv_host[k, cc, ck*128+m] = Wv[cc*128+k, perm(ck, m)]
    wv_host = np.empty((128, NCC, 512), np.float32)
    for ck in range(NCC):
        wv_host[:, :, ck * 128:(ck + 1) * 128] = (
            Wv[:, perm[ck]].reshape(NCC, 128, 128).transpose(1, 0, 2))
    # wd_host[b, k, cc, h] = Wd[b, cc*128+k, h]
    wd_host = Wd.reshape(B, NCC, 128, NH).transpose(0, 2, 1, 3)
    wd_host = np.ascontiguousarray(wd_host)
    # sel8[h, m] = 1 iff head(m) == h  (head(m) = m // 16)
    sel8 = np.zeros((NH, 128), np.float16)
    for hh in range(NH):
        sel8[hh, hh * 16:(hh + 1) * 16] = 1.0
    # wo_host[m, di, do*128+j] = Wout[perm(di, m), do*128+j]
    wo_host = np.empty((128, NCC, 512), np.float16)
    for di in range(NCC):
        wo_host[:, di, :] = Wout[perm[di], :]
    bout_host = np.ascontiguousarray(bout.reshape(NCC, 128).T)  # [k, do]

    in_maps = []
    for b in range(B):
        in_maps.append({
            "c": np.ascontiguousarray(c[b]),
            "wd": np.ascontiguousarray(wd_host[b]),
            "wv": wv_host,
            "sel8": sel8,
            "ones8": np.ones((128, NH), np.float32),
            "wout": wo_host,
            "bout": bout_host,
        })
    return in_maps


def kernel(**inputs) -> np.ndarray:
    nc = _get_nc()
    in_maps = _prep_inputs(**inputs)
    res = run_bass_kernel_spmd(nc, in_maps, list(range(B)))
    return np.stack([res.results[b]["out"] for b in range(B)],
                    axis=0).astype(np.float32)


if __name__ == "__main__":
    nc = build_program()
    print("program built ok")
